# revision 51
# baseline (speedup 1.0000x reference)
"""Causal self-attention Trainium2 kernel.

Reference (full): x[B=2,S=2048,D=1024] @ W_qkv + b_qkv -> 16-head causal
attention -> @ W_out + b_out.

Sharding: 8 cores = (batch b in 0..1) x (head-group hg in 0..3, 4 heads of
hd=64 each). Each core computes a partial output projection for its 4 heads
on its batch; the host sums the 4 head-group partials per batch (f16
partials, f32 accumulate) and adds the (constant) V-bias correction
bv @ W_out and b_out.

Device pipeline per core (data path in fp16; accumulation in fp32 PSUM;
softmax denominator in fp32/f32r):
  - x is pre-transposed on the host (numpy) and loaded per-span as plain
    DMAs -- no PE transposes, no DVE eviction copies, and q-span 0 lands
    ~4us after launch so compute starts immediately.
  - The dense 100%-utilization projection matmuls (QK^T proj, V proj,
    output proj) are chopped into ~1us pieces and interleaved between the
    ~50%-utilization attention chunks.  Long full-utilization bursts trip
    the PE activity monitor (HAM), which clamps the whole PE to half row
    rate for multi-us windows; smoothing the MAC activity keeps it mostly
    unclamped.  Pipeline: iteration qj runs attention for q-span qj while
    emitting QK proj for span qj+1 and the output projection for span
    qj-1 in the gaps; V proj for span qj runs at iteration start.
  - Attention per (span, head): scores transposed ST[k,q] (partial-N
    matmuls below the diagonal), exp on ACT with 1/8 scale, triangle mask
    on diagonal blocks, PV accumulates attnT plus a denominator row via a
    ones column in V.
  - Normalization: fast PSUM evict, K=1 f32r matmul broadcasts the
    denominator row, reciprocal_approx_fast, multiply on eviction; odd
    heads staged through SBUF and DMA'd to partitions 64..127.
"""
import numpy as np
from contextlib import ExitStack

import concourse.bacc as bacc
import concourse.tile as tile
from concourse import mybir
from concourse.bass_utils import run_bass_kernel_spmd

F32 = mybir.dt.float32
F32R = mybir.dt.float32r
F16 = mybir.dt.float16

B = 2
S = 2048
D = 1024
HD = 64
HG = 4            # head-groups (cores per batch)
HPG = 4           # heads per group
CL = HPG * HD     # 256 local head cols per core
P = 128
NDC = D // P      # 8 d-chunks
NQJ = S // 512    # 4 q-spans
NKC = S // P      # 16 k-chunks

_CACHED = {}


def _build():
    if "nc" in _CACHED:
        return _CACHED["nc"]
    nc = bacc.Bacc("TRN2", target_bir_lowering=False, debug=False)

    xt_d = nc.dram_tensor("xt", [P, NQJ * NDC * 512], F16,
                          kind="ExternalInput")
    wqk_d = nc.dram_tensor("wqk", [D, 2 * CL], F16, kind="ExternalInput")
    wv_d = nc.dram_tensor("wv", [D, CL], F16, kind="ExternalInput")
    wout_d = nc.dram_tensor("wout", [CL, D], F16, kind="ExternalInput")
    bqk_d = nc.dram_tensor("bqk", [P, 4], F32, kind="ExternalInput")
    tri_d = nc.dram_tensor("tri", [P, P], F16, kind="ExternalInput")
    ones_d = nc.dram_tensor("ones", [P, 68], F32, kind="ExternalInput")
    y_d = nc.dram_tensor("y", [S, D], F16, kind="ExternalOutput")

    with tile.TileContext(nc) as tc, ExitStack() as ctx:
        persist = ctx.enter_context(tc.tile_pool(name="persist", bufs=1))
        ptp = ctx.enter_context(tc.tile_pool(name="ptp", bufs=3))
        youtp = ctx.enter_context(tc.tile_pool(name="youtp", bufs=2))
        unp = ctx.enter_context(tc.tile_pool(name="unp", bufs=2))
        rcpp = ctx.enter_context(tc.tile_pool(name="rcpp", bufs=2))
        tmpp = ctx.enter_context(tc.tile_pool(name="tmpp", bufs=2))
        ps_sm = ctx.enter_context(tc.tile_pool(name="ps_sm", bufs=2, space="PSUM"))
        ps_st = ctx.enter_context(tc.tile_pool(name="ps_st", bufs=2, space="PSUM"))
        ps_av = ctx.enter_context(tc.tile_pool(name="ps_av", bufs=2, space="PSUM"))

        # ---- persistent tiles ----
        xT = persist.tile([P, NQJ, NDC, 512], F16, name="xT")       # 32KB/part
        qkt_sb = persist.tile([P, 4, S], F16, name="qkt_sb")        # 16KB/part
        v_sb = persist.tile([P, NKC, HPG, HD + 1], F16, name="v_sb")
        attnT = persist.tile([P, 2, S], F16, name="attnT")          # 8KB/part
        wout_sb = persist.tile([P, 2, D], F16, name="wout_sb")
        wqk_sb = persist.tile([P, NDC, 2 * CL], F16, name="wqk_sb")
        wv_sb = persist.tile([P, NDC, CL], F16, name="wv_sb")
        bqk_sb = persist.tile([P, 4], F32, name="bqk_sb")
        tri_sb = persist.tile([P, P], F16, name="tri_sb")
        ones_sb = persist.tile([P, 68], F32R, name="ones_sb")

        # x arrives transposed through the DMA XBAR: xT[p, qj, dc, qi] =
        # x[qj*512+qi, dc*128+p].  Spans 0,2 on the SP ring; weights and
        # spans 1,3 share the ACT ring, ordered so everything lands before
        # its first consumer.
        # x arrives pre-transposed AND pre-tiled from the host in the exact
        # SBUF layout, so each span is 128 contiguous 8KB runs (fast DMA)
        # and span 0 lands early enough to start compute immediately.
        def xspan(qj, eng):
            eng.dma_start(
                out=xT[:, qj],
                in_=xt_d.ap()[:, qj * NDC * 512:(qj + 1) * NDC * 512]
                .rearrange("p (c s) -> p c s", s=512))

        # The DMA engines round-robin across all queued instructions, so
        # the first-needed tensors (x span 0, wqk) are split into many
        # pieces to claim a proportionally larger bandwidth share and
        # complete first.
        for dc in range(NDC):
            nc.sync.dma_start(
                out=xT[:, 0, dc],
                in_=xt_d.ap()[:, dc * 512:(dc + 1) * 512])
        for qj in range(1, NQJ):
            xspan(qj, nc.sync)
        wqk_r = wqk_d.ap().rearrange("(c p) m -> p c m", p=P)
        wv_r = wv_d.ap().rearrange("(c p) m -> p c m", p=P)
        for mc in range(4):
            nc.scalar.dma_start(out=wqk_sb[:, :, mc * P:(mc + 1) * P],
                                in_=wqk_r[:, :, mc * P:(mc + 1) * P])
        for mh in range(2):
            nc.scalar.dma_start(out=wv_sb[:, :, mh * P:(mh + 1) * P],
                                in_=wv_r[:, :, mh * P:(mh + 1) * P])
        nc.scalar.dma_start(out=wout_sb, in_=wout_d.ap()
                            .rearrange("(c p) o -> p c o", p=P))
        # small constants on the SWDGE ring
        nc.gpsimd.dma_start(out=bqk_sb, in_=bqk_d.ap())
        nc.gpsimd.dma_start(out=tri_sb, in_=tri_d.ap())
        nc.gpsimd.dma_start(out=ones_sb, in_=ones_d.ap().bitcast(F32R))
        ones_row64 = ones_sb[64:65, 4:4 + HD]

        # ones column of V (denominator row accumulates via PV)
        nc.vector.memset(v_sb[:, :, :, HD], 1.0)

        # ---- dense (100%-util) projection pieces ----
        def qk_piece(qj, mc):
            q0 = qj * 512
            pq = ps_sm.tile([P, 512], F32, tag="sm", name=f"pq{qj}_{mc}")
            for kc in range(NDC):
                nc.tensor.matmul(
                    pq[:],
                    wqk_sb[:, kc, mc * P:(mc + 1) * P],
                    xT[:, qj, kc, :],
                    start=(kc == 0), stop=(kc == NDC - 1))
            nc.vector.tensor_scalar_add(
                qkt_sb[:, mc, q0:q0 + 512], pq[:], bqk_sb[:, mc:mc + 1])

        def v_piece(qj, si):
            sc = 4 * qj + si
            pv = ps_sm.tile([P, CL], F32, tag="sm", name=f"pv{sc}")
            for kc in range(NDC):
                nc.tensor.matmul(
                    pv[:],
                    xT[:, qj, kc, si * P:(si + 1) * P],
                    wv_sb[:, kc, :],
                    start=(kc == 0), stop=(kc == NDC - 1))
            nc.vector.tensor_copy(
                v_sb[:, sc, :, 0:HD],
                pv.rearrange("p (h d) -> p h d", h=HPG))

        def out_piece(qj, si, tail=False):
            # tail pieces run after all attention: borrow the idle ps_st
            # ring for 4 in-flight PSUM tiles and drain y per half
            sc = 4 * qj + si
            y_sb = youtp.tile([P, D], F16, tag="y", name=f"y{sc}")
            for oc in range(2):
                pool, tag = (ps_st, "st") if tail else (ps_sm, "sm")
                py = pool.tile([P, 512], F32, tag=tag,
                               name=f"py{sc}_{oc}")
                for cc in range(2):
                    nc.tensor.matmul(
                        py[:],
                        attnT[:, cc, sc * P:(sc + 1) * P],
                        wout_sb[:, cc, oc * 512:(oc + 1) * 512],
                        start=(cc == 0), stop=(cc == 1))
                # alternate engines so consecutive evictions overlap
                if oc == 0:
                    nc.vector.tensor_copy(
                        y_sb[:, oc * 512:(oc + 1) * 512], py[:])
                else:
                    nc.scalar.activation(
                        y_sb[:, oc * 512:(oc + 1) * 512], py[:],
                        mybir.ActivationFunctionType.Copy)
                if tail:
                    nc.sync.dma_start(
                        out=y_d.ap()[sc * P:(sc + 1) * P,
                                     oc * 512:(oc + 1) * 512],
                        in_=y_sb[:, oc * 512:(oc + 1) * 512])
            if not tail:
                nc.sync.dma_start(out=y_d.ap()[sc * P:(sc + 1) * P, :],
                                  in_=y_sb)

        # QK proj for span 0 must precede its attention
        for mc in range(4):
            qk_piece(0, mc)

        # ---- main pipeline over q-spans ----
        for qj in range(NQJ):
            q0 = qj * 512
            nkc = 4 * (qj + 1)

            # V proj for this span (PV below consumes it)
            for si in range(4):
                v_piece(qj, si)

            # dense work to sprinkle between this span's attention chunks
            dq = []
            if qj + 1 < NQJ:
                dq += [(qk_piece, (qj + 1, mc)) for mc in range(4)]
            if qj >= 1:
                dq += [(out_piece, (qj - 1, si)) for si in range(4)]
            # span 3 holds pieces back: they emit after the last head's
            # chunks and overlap its normalize chain before out_piece(3,*)
            den = 4 * (nkc // 2) + (5 if qj == NQJ - 1 else 0)
            done = 0
            emitted = 0

            def scores_chunk(h, pi):
                """Scores pair -> exp -> mask; returns the probs tile."""
                mck, pok = 2 + h // 2, 64 * (h % 2)
                mcq, poq = h // 2, 64 * (h % 2)
                stp = ps_st.tile([P, 1024], F32, tag="st",
                                 name=f"st{qj}_{h}_{pi}")
                pt = ptp.tile([P, 1024], F16, tag="pt",
                              name=f"pt{qj}_{h}_{pi}")
                for half in range(2):
                    kc = 2 * pi + half
                    t = kc - 4 * qj
                    c0 = 128 * t if t > 0 else 0
                    nc.tensor.matmul(
                        stp[:, 512 * half + c0: 512 * half + 512],
                        qkt_sb[pok:pok + 64, mck, kc * P:(kc + 1) * P],
                        qkt_sb[poq:poq + 64, mcq, q0 + c0: q0 + 512],
                        start=True, stop=True)
                t0 = 2 * pi - 4 * qj
                ec0 = 128 * t0 if t0 > 0 else 0
                c1 = 128 * (t0 + 1) if t0 + 1 > 0 else 0
                if c1 > 0:
                    # diagonal pair: skip the unwritten causal gap
                    nc.scalar.activation(
                        pt[:, ec0:512], stp[:, ec0:512],
                        mybir.ActivationFunctionType.Exp, scale=0.125)
                    nc.scalar.activation(
                        pt[:, 512 + c1:1024], stp[:, 512 + c1:1024],
                        mybir.ActivationFunctionType.Exp, scale=0.125)
                else:
                    nc.scalar.activation(
                        pt[:, ec0:1024], stp[:, ec0:1024],
                        mybir.ActivationFunctionType.Exp, scale=0.125)
                for half in range(2):
                    kc = 2 * pi + half
                    t = kc - 4 * qj
                    if 0 <= t <= 3:
                        off = 512 * half + 128 * t
                        nc.vector.tensor_mul(
                            pt[:, off:off + 128],
                            pt[:, off:off + 128], tri_sb)
                return pt

            def pv_chunk(h, pi, pt, av):
                for half in range(2):
                    kc = 2 * pi + half
                    t = kc - 4 * qj
                    c0 = 128 * t if t > 0 else 0
                    nc.tensor.matmul(
                        av[0:HD + 1, c0:512],
                        v_sb[:, kc, h, :],
                        pt[:, 512 * half + c0: 512 * half + 512],
                        start=(kc == 0), stop=(kc == nkc - 1))

            def make_normalize(h, av):
                # fast-evict av, then normalize in SBUF; odd heads staged
                # through SBUF and DMA'd to partitions 64..127
                def norm():
                    un = unp.tile([HD + 1, 512], F32R, tag="un",
                                  name=f"un{qj}_{h}")
                    nc.vector.tensor_copy(un, av[0:HD + 1, :])
                    dnb = ps_sm.tile([P, 512], F32, tag="sm",
                                     name=f"dnb{qj}_{h}")
                    nc.tensor.matmul(dnb[0:HD, :], ones_row64,
                                     un[HD:HD + 1, :],
                                     start=True, stop=True)
                    rbs = rcpp.tile([HD, 512], F32, tag="rbs",
                                    name=f"rbs{qj}_{h}")
                    nc.vector.reciprocal_approx_fast(rbs, dnb[0:HD, :])
                    c = h // 2
                    if h % 2 == 0:
                        nc.vector.tensor_mul(
                            attnT[0:HD, c, q0:q0 + 512], un[0:HD, :], rbs)
                    else:
                        tmp = tmpp.tile([HD, 512], F16, tag="tmp",
                                        name=f"tmp{qj}_{h}")
                        nc.vector.tensor_mul(tmp, un[0:HD, :], rbs)
                        nc.sync.dma_start(
                            out=attnT[HD:P, c, q0:q0 + 512], in_=tmp)
                return norm

            # Software-pipelined emission: PV for chunk k goes out after
            # the scores for chunk k+1, so the PE always has independent
            # matmuls to run while ACT computes the exp.  The previous
            # head's normalize chain is likewise deferred past the next
            # head's first scores chunk.
            pending_norm = None
            # odd heads first so their attnT partition-shift DMA hides
            for h in (1, 3, 0, 2):
                av = ps_av.tile([P, 512], F32, tag="av", name=f"av{qj}_{h}")
                prev_pt = None
                for pi in range(nkc // 2):
                    pt = scores_chunk(h, pi)
                    if prev_pt is not None:
                        pv_chunk(h, pi - 1, prev_pt, av)
                        if pi == 1 and pending_norm is not None:
                            # two chunks past the head boundary: the
                            # PSUM fast-evict has surely drained by now
                            pending_norm()
                            pending_norm = None
                    prev_pt = pt
                    # sprinkle dense pieces between attention chunks
                    done += 1
                    while emitted < min(len(dq), done * len(dq) // den):
                        f, a = dq[emitted]
                        f(*a)
                        emitted += 1
                pv_chunk(h, nkc // 2 - 1, prev_pt, av)
                pending_norm = make_normalize(h, av)

            # last head's normalize; leftover dense overlaps the chain
            pending_norm()
            for f, a in dq[emitted:]:
                f(*a)

        # output projection for the last span
        for si in range(4):
            out_piece(3, si, tail=True)

    nc.compile()
    _CACHED["nc"] = nc
    return nc


def _host_inputs(x, W_qkv, b_qkv):
    """Build the 8 per-core input maps (wout filled in by caller)."""
    x16 = np.asarray(x, dtype=np.float16)
    # [S, D] -> [p, qj, dc, qi] tile order matching the xT SBUF layout
    xt = [np.ascontiguousarray(
        x16[b].T.reshape(NDC, P, NQJ, 512).transpose(1, 2, 0, 3)
        .reshape(P, NQJ * NDC * 512)) for b in range(B)]
    tri = (np.arange(P)[None, :] >= np.arange(P)[:, None]).astype(np.float16)
    in_maps = []
    for b in range(B):
        for hg in range(HG):
            c0 = hg * CL
            wqk = np.ascontiguousarray(
                np.concatenate([W_qkv[:, c0:c0 + CL],
                                W_qkv[:, D + c0:D + c0 + CL]], axis=1)
                .astype(np.float16))
            wv = np.ascontiguousarray(
                W_qkv[:, 2 * D + c0:2 * D + c0 + CL].astype(np.float16))
            bqk = np.ascontiguousarray(
                np.concatenate([b_qkv[c0:c0 + CL],
                                b_qkv[D + c0:D + c0 + CL]])
                .reshape(4, P).T, dtype=np.float32)
            in_maps.append({
                "xt": xt[b], "wqk": wqk, "wv": wv, "wout": None,
                "bqk": bqk, "tri": tri,
                "ones": np.ones((P, 68), dtype=np.float32),
            })
    return in_maps


def kernel(x, W_qkv, b_qkv, W_out, b_out):
    x = np.asarray(x, dtype=np.float32)
    W_qkv = np.asarray(W_qkv, dtype=np.float32)
    b_qkv = np.asarray(b_qkv, dtype=np.float32)
    W_out = np.asarray(W_out, dtype=np.float32)
    b_out = np.asarray(b_out, dtype=np.float32)

    nc = _build()
    in_maps = _host_inputs(x, W_qkv, b_qkv)
    for i, m in enumerate(in_maps):
        hg = i % HG
        m["wout"] = np.ascontiguousarray(
            W_out[hg * CL:(hg + 1) * CL, :].astype(np.float16))
    core_ids = list(range(8))
    res = run_bass_kernel_spmd(nc, in_maps, core_ids)
    outs = [r["y"] for r in res.results]
    bv = b_qkv[2 * D:3 * D]
    corr = (bv @ W_out + b_out).astype(np.float32)
    y = np.empty((B, S, D), dtype=np.float32)
    for b in range(B):
        acc = outs[b * HG].astype(np.float32)
        for hg in range(1, HG):
            acc += outs[b * HG + hg].astype(np.float32)
        y[b] = acc + corr
    return y
